# Initial kernel scaffold
#
"""Trainium2 Bass kernel for nn_GCN1PoolNorm: 3-layer GCN + shared BatchNorm +
global max pool + MLP head, SPMD across 8 NeuronCores.

Self-contained: takes FULL inputs, returns FULL output [N_GRAPHS, N_CLASSES].

Design (per core = one 1/8 dst-shard of nodes) — DENSE aggregation:
- Message aggregation as a dense matmul: agg.T [64, nsh] = h_tilde.T @ A_c,
  where A_c [n_nodes, nsh] fp8 is the (0/1/multiplicity) adjacency column
  block for this core's dst shard, with self-loops on the diagonal. A_c is
  host-built once (graph is static), streamed from DRAM in bulk (fast HWDGE
  path), and reused by all 3 layers. This avoids the per-edge descriptor
  gather path entirely (~1us/descriptor on this runner's GPSIMD path).
- Table rows h_tilde = act * dis (dis = rsqrt(deg+1)) live in a Shared DRAM
  table [n_pad, 64] bf16, AllGather-published per layer; each layer loads the
  full table into SBUF as lhsT blocks [128, NB, 64].
- Per dst-range (512 cols): one PSUM bank accumulates NB chained matmuls
  (lhsT = table block bf16, rhs = A tile fp8). Downstream per node tile:
  U.T = psum * dis_rep; Z.T = W.T @ U.T; BN stats via ACT accum_out; stats
  AllReduce; BN affine+relu fused in one ACT op; * dis; PE transpose; DMA to
  table shard; AllGather.
- Pooling: graphs align exactly to cores; free-axis reduce_max segments;
  MLP head feat-major; out [gpc, 10] per core, host concatenates.
"""
import numpy as np
import ml_dtypes

from concourse import bacc, mybir, tile
from concourse.bass_utils import run_bass_kernel_spmd
from concourse.masks import make_identity

f32 = mybir.dt.float32
bf16 = mybir.dt.bfloat16
fp8 = mybir.dt.float8e4

N_CORES = 8
P = 128          # partition / block quantum
D = 64           # feature dim
RW = 512         # dst-range width (one PSUM bank)
BN_EPS = 1e-5


# ---------------------------------------------------------------- host prep

def _prep(x, edge_index, batch, n_classes):
    n_nodes = x.shape[0]
    n_graphs = int(batch.max()) + 1
    assert n_nodes % N_CORES == 0
    nsh = n_nodes // N_CORES                    # nodes per core
    ntile = (nsh + P - 1) // P                  # node tiles per core
    tsz = [min(P, nsh - t * P) for t in range(ntile)]
    NB = (n_nodes + P - 1) // P                 # src blocks (padded rows)
    npad = NB * P
    R = (nsh + RW - 1) // RW                    # dst ranges per core
    rsz = [min(RW, nsh - r * RW) for r in range(R)]

    src = edge_index[0].astype(np.int64)
    dst = edge_index[1].astype(np.int64)
    deg = np.bincount(dst, minlength=n_nodes).astype(np.int64)

    # A[c]: [R, NB, 128, RW] fp8; entry (r, b, s, d) = multiplicity of edge
    # (128b+s -> c*nsh + r*RW + d), plus self loops.
    A = []
    core_of = dst // nsh
    dloc = dst % nsh
    for c in range(N_CORES):
        m = core_of == c
        Ac = np.zeros((NB * P, nsh), np.uint8)
        np.add.at(Ac, (src[m], dloc[m]), 1)
        sn = np.arange(c * nsh, (c + 1) * nsh)
        Ac[sn, np.arange(nsh)] += 1             # self loops
        # p-major layout: per-partition contiguous 8KB runs in the chunk DMA
        Af = np.zeros((R, P, NB, RW), ml_dtypes.float8_e4m3)
        for r in range(R):
            blk = Ac[:, r * RW:r * RW + rsz[r]].reshape(NB, P, rsz[r])
            Af[r, :, :, :rsz[r]] = blk.transpose(1, 0, 2).astype(
                ml_dtypes.float8_e4m3)
        A.append(Af)
        del Ac

    # deg layouts (fp32)
    deg_pt = np.zeros((N_CORES, P, ntile), np.float32)
    deg_row = np.zeros((N_CORES, 1, nsh), np.float32)
    for c in range(N_CORES):
        dsh = deg[c * nsh:(c + 1) * nsh].astype(np.float32)
        deg_row[c, 0, :] = dsh
        for t in range(ntile):
            deg_pt[c, :tsz[t], t] = dsh[t * P:t * P + tsz[t]]

    # pooling segments (identical across cores required for SPMD)
    gb = np.searchsorted(batch, np.arange(n_graphs + 1))
    gpc = n_graphs // N_CORES
    loc0 = gb[:gpc + 1].copy()
    for c in range(N_CORES):
        locc = gb[c * gpc:(c + 1) * gpc + 1] - c * nsh
        assert np.array_equal(locc, loc0), "graph pattern must match across cores"
    pool_segs = []
    for t in range(ntile):
        a, b = t * P, t * P + tsz[t]
        for g in range(gpc):
            s, e = max(a, int(loc0[g])), min(b, int(loc0[g + 1]))
            if s < e:
                pool_segs.append((t, s - a, e - a, g))

    cfg = dict(n_nodes=n_nodes, npad=npad, NB=NB, nsh=nsh, ntile=ntile,
               tsz=tsz, R=R, rsz=rsz, pool_segs=pool_segs, gpc=gpc,
               n_classes=n_classes, n_graphs=n_graphs)
    data = dict(A=A, deg_pt=deg_pt, deg_row=deg_row)
    return cfg, data


# ---------------------------------------------------------------- device build

def _build(cfg, reps=1):
    nsh, ntile, tsz = cfg["nsh"], cfg["ntile"], cfg["tsz"]
    NB, npad = cfg["NB"], cfg["npad"]
    R, rsz = cfg["R"], cfg["rsz"]
    ncls, gpc = cfg["n_classes"], cfg["gpc"]
    n_nodes = cfg["n_nodes"]
    nshp = ntile * P
    BC = 16                                     # A blocks per stream chunk

    nc = bacc.Bacc(trn_type="TRN2", target_bir_lowering=False, debug=False,
                   num_devices=N_CORES)

    x_sh = nc.dram_tensor("x_sh", [nsh, D], f32, kind="ExternalInput").ap()
    A_in = nc.dram_tensor("A", [R, P, NB, RW], fp8, kind="ExternalInput").ap()
    deg_pt = nc.dram_tensor("deg_pt", [P, ntile], f32, kind="ExternalInput").ap()
    deg_row = nc.dram_tensor("deg_row", [1, nsh], f32, kind="ExternalInput").ap()
    Ws = [nc.dram_tensor(f"W{i}", [D, D], bf16, kind="ExternalInput").ap()
          for i in (1, 2, 3)]
    gamma = nc.dram_tensor("gamma", [D, 1], f32, kind="ExternalInput").ap()
    beta = nc.dram_tensor("beta", [D, 1], f32, kind="ExternalInput").ap()
    lin1w = nc.dram_tensor("lin1w", [D, D], bf16, kind="ExternalInput").ap()
    lin1b = nc.dram_tensor("lin1b", [D, 1], f32, kind="ExternalInput").ap()
    lin2w = nc.dram_tensor("lin2w", [D, ncls], bf16, kind="ExternalInput").ap()
    lin2b = nc.dram_tensor("lin2b", [ncls, 1], f32, kind="ExternalInput").ap()
    out = nc.dram_tensor("out", [gpc, ncls], f32, kind="ExternalOutput").ap()

    table = nc.dram_tensor("table", [npad, D], bf16, addr_space="Shared").ap()
    tshard = nc.dram_tensor("tshard", [nsh, D], bf16).ap()
    stats_in = nc.dram_tensor("stats_in", [D, 2], f32).ap()
    stats_out = nc.dram_tensor("stats_out", [D, 2], f32,
                               addr_space="Shared").ap()

    with tile.TileContext(nc) as tc:
        with (
            tc.tile_pool(name="const", bufs=1) as cpool,
            tc.tile_pool(name="abuf", bufs=3) as apool,
            tc.tile_pool(name="work", bufs=3) as wpool,
            tc.tile_pool(name="psacc", bufs=2, space="PSUM") as ps_acc,
            tc.tile_pool(name="psz", bufs=2, space="PSUM") as ps_z,
            tc.tile_pool(name="pstr", bufs=2, space="PSUM") as ps_tr,
        ):
            # ---- residents
            tabsb = cpool.tile([P, NB, D], bf16)          # full table (lhsT blocks)
            dis_pt = cpool.tile([P, ntile], f32)
            dis_rep = cpool.tile([D, nshp], bf16)
            zbuf = cpool.tile([D, nshp], bf16)
            act3 = zbuf
            sums = cpool.tile([D, ntile], f32)
            sums2 = cpool.tile([D, ntile], f32)
            W_sb = [cpool.tile([D, D], bf16, tag=f"W{i}", name=f"W{i}_sb")
                    for i in range(3)]
            for i in range(3):
                nc.sync.dma_start(out=W_sb[i][:], in_=Ws[i][:])
            gamma_sb = cpool.tile([D, 1], f32, tag="gamma")
            beta_sb = cpool.tile([D, 1], f32, tag="beta")
            nc.sync.dma_start(out=gamma_sb[:], in_=gamma[:])
            nc.sync.dma_start(out=beta_sb[:], in_=beta[:])
            l1w_sb = cpool.tile([D, D], bf16, tag="l1w")
            l1b_sb = cpool.tile([D, 1], f32, tag="l1b")
            l2w_sb = cpool.tile([D, ncls], bf16, tag="l2w")
            l2b_sb = cpool.tile([ncls, 1], f32, tag="l2b")
            nc.sync.dma_start(out=l1w_sb[:], in_=lin1w[:])
            nc.sync.dma_start(out=l1b_sb[:], in_=lin1b[:])
            nc.sync.dma_start(out=l2w_sb[:], in_=lin2w[:])
            nc.sync.dma_start(out=l2b_sb[:], in_=lin2b[:])
            ident = cpool.tile([D, D], bf16, tag="ident")
            make_identity(nc, ident[:])
            ones1 = cpool.tile([1, D], bf16, tag="ones1")
            nc.gpsimd.memset(ones1[:], 1.0)
            emb = cpool.tile([D, gpc], f32, tag="emb")
            eps_sb = cpool.tile([D, 1], f32, tag="eps")
            nc.gpsimd.memset(eps_sb[:], BN_EPS)

            # zero the table pad rows once (streamed into tabsb; NaN-unsafe)
            if npad > n_nodes:
                zpad = wpool.tile([P, D], bf16, tag="zpad")
                nc.gpsimd.memset(zpad[:], 0.0)
                nc.sync.dma_start(out=table[n_nodes:npad, :],
                                  in_=zpad[:npad - n_nodes, :])

            # ---- dis
            dptf = wpool.tile([P, ntile], f32, tag="dptf")
            nc.sync.dma_start(out=dptf[:], in_=deg_pt[:])
            nc.scalar.activation(dis_pt[:], dptf[:],
                                 mybir.ActivationFunctionType.Sqrt, bias=1.0)
            nc.vector.reciprocal(dis_pt[:], dis_pt[:])
            for o in range(0, nsh, RW):
                w = min(RW, nsh - o)
                dsl = wpool.tile([1, RW], f32, tag="dsl")
                nc.sync.dma_start(out=dsl[:, :w], in_=deg_row[:, o:o + w])
                nc.scalar.activation(dsl[:, :w], dsl[:, :w],
                                     mybir.ActivationFunctionType.Sqrt, bias=1.0)
                nc.vector.reciprocal(dsl[:, :w], dsl[:, :w])
                dslb = wpool.tile([1, RW], bf16, tag="dslb")
                nc.vector.tensor_copy(dslb[:, :w], dsl[:, :w])
                pb = ps_z.tile([D, RW], f32, tag="zt", space="PSUM")
                nc.tensor.matmul(pb[:, :w], lhsT=ones1[:], rhs=dslb[:, :w],
                                 start=True, stop=True)
                nc.vector.tensor_copy(dis_rep[:, o:o + w], pb[:, :w])

            # ---- table0 = bf16(x * dis)
            for t in range(ntile):
                w = tsz[t]
                xt = wpool.tile([P, D], f32, tag="xt")
                nc.sync.dma_start(out=xt[:w, :], in_=x_sh[t * P:t * P + w, :])
                xb = wpool.tile([P, D], bf16, tag="xb")
                nc.scalar.activation(xb[:w, :], xt[:w, :],
                                     mybir.ActivationFunctionType.Copy,
                                     scale=dis_pt[:w, t:t + 1])
                nc.sync.dma_start(out=tshard[t * P:t * P + w, :], in_=xb[:w, :])
            nc.gpsimd.collective_compute(
                "AllGather", mybir.AluOpType.bypass,
                replica_groups=[list(range(N_CORES))],
                ins=[tshard[:, :].opt()], outs=[table[:n_nodes, :].opt()])

            # ---- layers
            for rep in range(reps):
                for li in range(3):
                    last = (li == 2)
                    Wl = W_sb[li]
                    # load the full table as lhsT blocks [s, b, :]
                    # (split: <=32 blocks per DMA keeps descriptor count <=4096)
                    tview = table.rearrange("(b s) d -> s b d", s=P)
                    for b0 in range(0, NB, 32):
                        bw = min(32, NB - b0)
                        nc.sync.dma_start(out=tabsb[:, b0:b0 + bw, :],
                                          in_=tview[:, b0:b0 + bw, :])
                    for r in range(R):
                        rw = rsz[r]
                        ps = ps_acc.tile([D, RW], f32, tag="acc", space="PSUM")
                        for b0 in range(0, NB, BC):
                            bw = min(BC, NB - b0)
                            asb = apool.tile([P, BC, RW], fp8, tag="A")
                            nc.sync.dma_start(
                                out=asb[:, :bw, :],
                                in_=A_in[r, :, b0:b0 + bw, :])
                            for bi in range(bw):
                                b = b0 + bi
                                nc.tensor.matmul(
                                    ps[:, :rw], lhsT=tabsb[:, b, :],
                                    rhs=asb[:, bi, :rw],
                                    start=(b == 0), stop=(b == NB - 1))
                        # downstream per node tile in this range
                        for tt in range((rw + P - 1) // P):
                            t = (r * RW) // P + tt
                            w = tsz[t]
                            psl = ps[:, tt * P:tt * P + w]
                            u2t = wpool.tile([D, P], bf16, tag="u2t")
                            nc.vector.tensor_tensor(
                                out=u2t[:, :w], in0=psl,
                                in1=dis_rep[:, t * P:t * P + w],
                                op=mybir.AluOpType.mult)
                            psz = ps_z.tile([D, P], f32, tag="zt", space="PSUM")
                            nc.tensor.matmul(psz[:, :w], lhsT=Wl[:],
                                             rhs=u2t[:, :w],
                                             start=True, stop=True)
                            zslice = (act3 if last else zbuf)[:, t * P:t * P + w]
                            nc.scalar.activation(
                                zslice, psz[:, :w],
                                mybir.ActivationFunctionType.Copy,
                                accum_out=sums[:, t:t + 1])
                            sq = wpool.tile([D, P], f32, tag="sq")
                            nc.scalar.activation(
                                sq[:, :w], psz[:, :w],
                                mybir.ActivationFunctionType.Square,
                                accum_out=sums2[:, t:t + 1])

                    # ---- global BN stats
                    st = wpool.tile([D, 2], f32, tag="st")
                    nc.vector.reduce_sum(st[:, 0:1], sums[:],
                                         axis=mybir.AxisListType.X)
                    nc.vector.reduce_sum(st[:, 1:2], sums2[:],
                                         axis=mybir.AxisListType.X)
                    nc.sync.dma_start(out=stats_in[:], in_=st[:])
                    nc.gpsimd.collective_compute(
                        "AllReduce", mybir.AluOpType.add,
                        replica_groups=[list(range(N_CORES))],
                        ins=[stats_in[:, :].opt()], outs=[stats_out[:, :].opt()])
                    stg = wpool.tile([D, 2], f32, tag="stg")
                    nc.sync.dma_start(out=stg[:], in_=stats_out[:])
                    mu = wpool.tile([D, 1], f32, tag="mu")
                    nc.scalar.activation(mu[:], stg[:, 0:1],
                                         mybir.ActivationFunctionType.Copy,
                                         scale=1.0 / n_nodes)
                    va = wpool.tile([D, 1], f32, tag="va")
                    nc.scalar.activation(va[:], stg[:, 1:2],
                                         mybir.ActivationFunctionType.Copy,
                                         scale=1.0 / n_nodes)
                    mu2 = wpool.tile([D, 1], f32, tag="mu2")
                    nc.vector.tensor_tensor(out=mu2[:], in0=mu[:], in1=mu[:],
                                            op=mybir.AluOpType.mult)
                    nc.vector.tensor_tensor(out=va[:], in0=va[:], in1=mu2[:],
                                            op=mybir.AluOpType.subtract)
                    nc.scalar.activation(va[:], va[:],
                                         mybir.ActivationFunctionType.Sqrt,
                                         bias=eps_sb[:])
                    nc.vector.reciprocal(va[:], va[:])
                    saff = wpool.tile([D, 1], f32, tag="saff")
                    nc.vector.tensor_tensor(out=saff[:], in0=gamma_sb[:],
                                            in1=va[:], op=mybir.AluOpType.mult)
                    tsh_ = wpool.tile([D, 1], f32, tag="tsh")
                    nc.vector.tensor_tensor(out=tsh_[:], in0=mu[:], in1=saff[:],
                                            op=mybir.AluOpType.mult)
                    nc.vector.tensor_tensor(out=tsh_[:], in0=beta_sb[:],
                                            in1=tsh_[:],
                                            op=mybir.AluOpType.subtract)

                    # ---- activation phase
                    for t in range(ntile):
                        w = tsz[t]
                        zsl = (act3 if last else zbuf)[:, t * P:t * P + w]
                        at = wpool.tile([D, P], bf16, tag="at")
                        nc.scalar.activation(at[:, :w], zsl,
                                             mybir.ActivationFunctionType.Relu,
                                             bias=tsh_[:], scale=saff[:])
                        if not last:
                            ht = wpool.tile([D, P], bf16, tag="ht")
                            nc.vector.tensor_tensor(
                                out=ht[:, :w], in0=at[:, :w],
                                in1=dis_rep[:, t * P:t * P + w],
                                op=mybir.AluOpType.mult)
                            ptr = ps_tr.tile([P, D], bf16, tag="tr",
                                             space="PSUM")
                            nc.tensor.transpose(ptr[:w, :], ht[:, :w],
                                                ident[:, :])
                            wr = wpool.tile([P, D], bf16, tag="wr")
                            nc.vector.tensor_copy(wr[:w, :], ptr[:w, :])
                            nc.sync.dma_start(
                                out=tshard[t * P:t * P + w, :], in_=wr[:w, :])
                        else:
                            nc.vector.tensor_copy(act3[:, t * P:t * P + w],
                                                  at[:, :w])
                    if not last:
                        nc.gpsimd.collective_compute(
                            "AllGather", mybir.AluOpType.bypass,
                            replica_groups=[list(range(N_CORES))],
                            ins=[tshard[:, :].opt()],
                            outs=[table[:n_nodes, :].opt()])

            # ---- pooling
            first_seen = set()
            for (t, s0, s1, g) in cfg["pool_segs"]:
                tmp = wpool.tile([D, 1], f32, tag="ptmp")
                nc.vector.reduce_max(tmp[:], act3[:, t * P + s0:t * P + s1],
                                     axis=mybir.AxisListType.X)
                if g not in first_seen:
                    first_seen.add(g)
                    nc.vector.tensor_copy(emb[:, g:g + 1], tmp[:])
                else:
                    nc.vector.tensor_tensor(out=emb[:, g:g + 1],
                                            in0=emb[:, g:g + 1], in1=tmp[:],
                                            op=mybir.AluOpType.max)

            # ---- head
            emb_bf = wpool.tile([D, gpc], bf16, tag="embbf")
            nc.vector.tensor_copy(emb_bf[:], emb[:])
            ph = ps_z.tile([D, gpc], f32, tag="zt", space="PSUM")
            nc.tensor.matmul(ph[:], lhsT=l1w_sb[:], rhs=emb_bf[:],
                             start=True, stop=True)
            h1 = wpool.tile([D, gpc], bf16, tag="h1")
            nc.scalar.activation(h1[:], ph[:],
                                 mybir.ActivationFunctionType.Relu,
                                 bias=l1b_sb[:])
            po = ps_tr.tile([ncls, gpc], f32, tag="tr", space="PSUM")
            nc.tensor.matmul(po[:], lhsT=l2w_sb[:], rhs=h1[:],
                             start=True, stop=True)
            osb = wpool.tile([ncls, gpc], f32, tag="osb")
            nc.scalar.activation(osb[:], po[:],
                                 mybir.ActivationFunctionType.Identity,
                                 bias=l2b_sb[:])
            nc.sync.dma_start(out=out[:, :].rearrange("g c -> c g"), in_=osb[:])

    nc.compile()
    return nc


# ---------------------------------------------------------------- entry point

_CACHE = {}


def _get_built(cfg_key, cfg, reps):
    key = (cfg_key, reps)
    if key not in _CACHE:
        _CACHE[key] = _build(cfg, reps=reps)
    return _CACHE[key]


def kernel(x, edge_index, batch, W1, b1, W2, b2, W3, b3, gamma, beta,
           lin1_w, lin1_b, lin2_w, lin2_b, _reps=1):
    x = np.asarray(x, np.float32)
    edge_index = np.asarray(edge_index)
    batch = np.asarray(batch)
    n_nodes, d = x.shape
    ncls = np.asarray(lin2_w).shape[1]
    assert d == D

    cfg, data = _prep(x, edge_index, batch, ncls)
    nsh = cfg["nsh"]

    # NOTE: b1/b2/b3 cancel inside BatchNorm (mean subtraction) - unused.
    W_bf = [np.asarray(w, np.float32).astype(ml_dtypes.bfloat16)
            for w in (W1, W2, W3)]
    in_maps = []
    for c in range(N_CORES):
        in_maps.append({
            "x_sh": x[c * nsh:(c + 1) * nsh].astype(np.float32),
            "A": data["A"][c],
            "deg_pt": data["deg_pt"][c],
            "deg_row": data["deg_row"][c],
            "W1": W_bf[0], "W2": W_bf[1], "W3": W_bf[2],
            "gamma": np.asarray(gamma, np.float32).reshape(D, 1),
            "beta": np.asarray(beta, np.float32).reshape(D, 1),
            "lin1w": np.asarray(lin1_w, np.float32).astype(ml_dtypes.bfloat16),
            "lin1b": np.asarray(lin1_b, np.float32).reshape(D, 1),
            "lin2w": np.asarray(lin2_w, np.float32).astype(ml_dtypes.bfloat16),
            "lin2b": np.asarray(lin2_b, np.float32).reshape(ncls, 1),
        })

    cfg_key = (n_nodes, edge_index.shape[1], ncls)
    nc = _get_built(cfg_key, cfg, _reps)
    res = run_bass_kernel_spmd(nc, in_maps, core_ids=list(range(N_CORES)))
    outs = [res.results[c]["out"] for c in range(N_CORES)]
    return np.concatenate(outs, axis=0).astype(np.float32)



# revision 4
# speedup vs baseline: 1.0129x; 1.0129x over previous
"""Trainium2 Bass kernel for nn_GCN1PoolNorm: 3-layer GCN + shared BatchNorm +
global max pool + MLP head.

Self-contained: takes FULL inputs, returns FULL output [N_GRAPHS, N_CLASSES].

v3 design — FULLY REPLICATED, ZERO COLLECTIVES:
On this runner every collective costs ~27-38 ms (software-emulated NRT), so
any sharded design is collective-bound. Instead every core runs the whole
graph; the host takes core 0's output. No cross-core traffic at all.

Per layer (full graph, N=50000 nodes, E=850k edges incl. self loops):
- Node features h_tilde = act * dis live in a local DRAM table [N, 128] bf16
  (cols 0:64 = payload, 64:128 = zero pad -> 256B rows for SWDGE dma_gather).
- Host sorts edges by (dst window of 128, src half, src), pads each
  (window, half) group to 128-edge blocks. Per chunk of WC windows:
  2 dma_gathers (int16 idx limit -> src < 32768 and rest) fetch h_tilde[src]
  rows edge-major; S one-hot blocks [128e, 128d] fp8 stream from DRAM.
- Aggregation per window: chained PE matmuls psum[128d, 64f] +=
  S_blk.T @ msgs_blk[:, 0:64]  (node-major).
- Downstream per 4-window group: U = ACT(psum) * dis (per-partition scale),
  PE transpose to feat-major, Z = W.T @ U, BN stats via ACT accum_out
  (locally -> global stats without any AllReduce), z spilled to DRAM.
- Act phase per group: z -> BN affine + relu -> (layers 1,2) transpose back,
  * dis, write table rows; (layer 3) pool graph segments on the fly.
- Head computed for all 64 graphs on every core.
"""
import numpy as np
import ml_dtypes

from concourse import bacc, mybir, tile
from concourse.bass_utils import run_bass_kernel_spmd
from concourse.masks import make_identity

f32 = mybir.dt.float32
bf16 = mybir.dt.bfloat16
fp8 = mybir.dt.float8e4
i16 = mybir.dt.int16

N_CORES = 8
P = 128          # partition / block / dst-window quantum
D = 64           # feature dim
HALF = 32768     # int16 gather index limit
TROW = 128       # table row width (64 feats + 64 zero pad) -> 256B rows
BN_EPS = 1e-5
WC = 8           # dst windows per stream chunk
GW = 4           # windows per downstream group (512 nodes)


# ---------------------------------------------------------------- host prep

def _prep(x, edge_index, batch, n_classes):
    n_nodes = x.shape[0]
    n_graphs = int(batch.max()) + 1
    ntile = (n_nodes + P - 1) // P              # dst windows
    tsz = [min(P, n_nodes - t * P) for t in range(ntile)]

    src = edge_index[0].astype(np.int64)
    dst = edge_index[1].astype(np.int64)
    deg = np.bincount(dst, minlength=n_nodes).astype(np.int64)

    # edge stream (incl. self loops), sorted by (window, half, src)
    s = np.concatenate([src, np.arange(n_nodes)])
    dl = np.concatenate([dst, np.arange(n_nodes)])
    t = dl // P
    h = (s >= HALF).astype(np.int64)
    col = dl % P
    order = np.lexsort((s, h, t))
    s, t, h, col = s[order], t[order], h[order], col[order]
    cnts = np.zeros((ntile, 2), np.int64)
    np.add.at(cnts, (t, h), 1)

    nb = (cnts + P - 1) // P                    # [ntile, 2] blocks per group
    NB0 = int(nb[:, 0].sum())
    NB1 = int(nb[:, 1].sum())
    NBLK = NB0 + NB1
    cum0 = np.concatenate([[0], np.cumsum(nb[:, 0])]).astype(int)
    cum1 = np.concatenate([[0], np.cumsum(nb[:, 1])]).astype(int)
    sb = np.concatenate([[0], np.cumsum(nb.sum(axis=1))]).astype(int)

    idx0 = np.zeros(NB0 * P, np.int16)
    idx1 = np.zeros(NB1 * P, np.int16)
    S = np.zeros((P, NBLK, P), ml_dtypes.float8_e4m3)
    off = 0
    for tt in range(ntile):
        for hh in (0, 1):
            n = int(cnts[tt, hh])
            e = slice(off, off + n)
            off += n
            q = np.arange(n)
            if hh == 0:
                idx0[cum0[tt] * P:cum0[tt] * P + n] = s[e]
                blk0 = sb[tt]
            else:
                idx1[cum1[tt] * P:cum1[tt] * P + n] = s[e] - HALF
                blk0 = sb[tt] + nb[tt, 0]
            S[q % P, blk0 + q // P, col[e]] = 1.0
    # wrap indices: idx g -> [g % 16, g // 16], replicated on 128 partitions
    idx0w = np.tile(idx0.reshape(-1, 16).T, (8, 1))
    idx1w = np.tile(idx1.reshape(-1, 16).T, (8, 1))
    # pack per chunk: [idx0_chunk | idx1_chunk] contiguous -> 1 DMA per chunk
    chunks = [(t0, min(t0 + WC, ntile)) for t0 in range(0, ntile, WC)]
    segs = []
    for (t0, t1) in chunks:
        segs.append(idx0w[:, cum0[t0] * 8:cum0[t1] * 8])
        segs.append(idx1w[:, cum1[t0] * 8:cum1[t1] * 8])
    idxc = np.concatenate(segs, axis=1).copy()

    # deg layouts (fp32)
    deg_pt = np.zeros((P, ntile), np.float32)
    degf = deg.astype(np.float32)
    for tt in range(ntile):
        deg_pt[:tsz[tt], tt] = degf[tt * P:tt * P + tsz[tt]]

    # pooling segments grouped by 512-node groups (for on-the-fly pooling)
    gb = np.searchsorted(batch, np.arange(n_graphs + 1))
    ngrp = (ntile + GW - 1) // GW
    pool_segs = []                              # (grp, s0, s1, g) rel to group
    for gr in range(ngrp):
        a, b = gr * GW * P, min((gr + 1) * GW * P, n_nodes)
        for g in range(n_graphs):
            s0, e0 = max(a, int(gb[g])), min(b, int(gb[g + 1]))
            if s0 < e0:
                pool_segs.append((gr, s0 - a, e0 - a, g))

    cfg = dict(n_nodes=n_nodes, ntile=ntile, tsz=tsz,
               nb=nb.tolist(), NB0=NB0, NB1=NB1, NBLK=NBLK,
               cum0=cum0.tolist(), cum1=cum1.tolist(), sb=sb.tolist(),
               pool_segs=pool_segs, ngrp=ngrp,
               n_classes=n_classes, n_graphs=n_graphs)
    data = dict(idxc=idxc, S=S, deg_pt=deg_pt)
    return cfg, data


# ---------------------------------------------------------------- device build

def _build(cfg, reps=1, n_devices=N_CORES):
    ntile, tsz = cfg["ntile"], cfg["tsz"]
    nb, NB0, NB1, NBLK = cfg["nb"], cfg["NB0"], cfg["NB1"], cfg["NBLK"]
    cum0, cum1, sb = cfg["cum0"], cfg["cum1"], cfg["sb"]
    ncls = cfg["n_classes"]
    n_nodes, n_graphs = cfg["n_nodes"], cfg["n_graphs"]
    ngrp = cfg["ngrp"]
    ntp = ntile * P
    RW = 512

    chunks = [(t0, min(t0 + WC, ntile)) for t0 in range(0, ntile, WC)]
    CB0 = max(cum0[t1] - cum0[t0] for t0, t1 in chunks)
    CB1 = max(cum1[t1] - cum1[t0] for t0, t1 in chunks)
    CBS = max(sb[t1] - sb[t0] for t0, t1 in chunks)

    nc = bacc.Bacc(trn_type="TRN2", target_bir_lowering=False, debug=False,
                   num_devices=n_devices)

    x_in = nc.dram_tensor("x", [n_nodes, D], f32, kind="ExternalInput").ap()
    S_in = nc.dram_tensor("S", [P, NBLK, P], fp8, kind="ExternalInput").ap()
    idxc_in = nc.dram_tensor("idxc", [P, (NB0 + NB1) * 8], i16,
                             kind="ExternalInput").ap()
    deg_pt = nc.dram_tensor("deg_pt", [P, ntile], f32, kind="ExternalInput").ap()
    Ws = [nc.dram_tensor(f"W{i}", [D, D], bf16, kind="ExternalInput").ap()
          for i in (1, 2, 3)]
    gamma = nc.dram_tensor("gamma", [D, 1], f32, kind="ExternalInput").ap()
    beta = nc.dram_tensor("beta", [D, 1], f32, kind="ExternalInput").ap()
    lin1w = nc.dram_tensor("lin1w", [D, D], bf16, kind="ExternalInput").ap()
    lin1b = nc.dram_tensor("lin1b", [D, 1], f32, kind="ExternalInput").ap()
    lin2w = nc.dram_tensor("lin2w", [D, ncls], bf16, kind="ExternalInput").ap()
    lin2b = nc.dram_tensor("lin2b", [ncls, 1], f32, kind="ExternalInput").ap()
    out = nc.dram_tensor("out", [n_graphs, ncls], f32,
                         kind="ExternalOutput").ap()

    table = nc.dram_tensor("table", [ntp, TROW], bf16).ap()
    z_dram = nc.dram_tensor("z_dram", [D, ntp], bf16).ap()

    with tile.TileContext(nc) as tc:
        with (
            tc.tile_pool(name="const", bufs=1) as cpool,
            tc.tile_pool(name="sbuf_s", bufs=2) as spool,
            tc.tile_pool(name="msgs", bufs=2) as mpool,
            tc.tile_pool(name="idxp", bufs=2) as ipool,
            tc.tile_pool(name="work", bufs=3) as wpool,
            tc.tile_pool(name="ump", bufs=6) as umpool,
            tc.tile_pool(name="psacc", bufs=2, space="PSUM") as ps_acc,
            tc.tile_pool(name="pstru", bufs=2, space="PSUM") as ps_tru,
            tc.tile_pool(name="psz", bufs=2, space="PSUM") as ps_z,
        ):
            # ---- residents
            dis_pt = cpool.tile([P, ntile], f32)
            sums = cpool.tile([D, ngrp], f32)
            sums2 = cpool.tile([D, ngrp], f32)
            W_sb = [cpool.tile([D, D], bf16, tag=f"W{i}", name=f"W{i}_sb")
                    for i in range(3)]
            for i in range(3):
                nc.sync.dma_start(out=W_sb[i][:], in_=Ws[i][:])
            gamma_sb = cpool.tile([D, 1], f32, tag="gamma")
            beta_sb = cpool.tile([D, 1], f32, tag="beta")
            nc.sync.dma_start(out=gamma_sb[:], in_=gamma[:])
            nc.sync.dma_start(out=beta_sb[:], in_=beta[:])
            l1w_sb = cpool.tile([D, D], bf16, tag="l1w")
            l1b_sb = cpool.tile([D, 1], f32, tag="l1b")
            l2w_sb = cpool.tile([D, ncls], bf16, tag="l2w")
            l2b_sb = cpool.tile([ncls, 1], f32, tag="l2b")
            nc.sync.dma_start(out=l1w_sb[:], in_=lin1w[:])
            nc.sync.dma_start(out=l1b_sb[:], in_=lin1b[:])
            nc.sync.dma_start(out=l2w_sb[:], in_=lin2w[:])
            nc.sync.dma_start(out=l2b_sb[:], in_=lin2b[:])
            identP = cpool.tile([P, P], bf16, tag="identP")
            make_identity(nc, identP[:])
            emb = cpool.tile([D, n_graphs], f32, tag="emb")
            eps_sb = cpool.tile([D, 1], f32, tag="eps")
            nc.gpsimd.memset(eps_sb[:], BN_EPS)

            # one-time zero of table right half + pad rows
            zpad = cpool.tile([P, TROW], bf16, tag="zpad")
            nc.gpsimd.memset(zpad[:], 0.0)
            for tt in range(ntile):
                w = tsz[tt]
                nc.sync.dma_start(out=table[tt * P:tt * P + w, D:TROW],
                                  in_=zpad[:w, :D])
                if w < P:
                    nc.sync.dma_start(out=table[tt * P + w:(tt + 1) * P, :],
                                      in_=zpad[:P - w, :])

            # ---- dis (node-major per-partition layout only)
            dptf = wpool.tile([P, ntile], f32, tag="dptf")
            nc.sync.dma_start(out=dptf[:], in_=deg_pt[:])
            nc.scalar.activation(dis_pt[:], dptf[:],
                                 mybir.ActivationFunctionType.Sqrt, bias=1.0)
            nc.vector.reciprocal(dis_pt[:], dis_pt[:])

            # ---- table0 = bf16(x * dis)
            for tt in range(ntile):
                w = tsz[tt]
                xt = wpool.tile([P, D], f32, tag="xt")
                nc.sync.dma_start(out=xt[:w, :], in_=x_in[tt * P:tt * P + w, :])
                xb = wpool.tile([P, D], bf16, tag="xb")
                nc.scalar.activation(xb[:w, :], xt[:w, :],
                                     mybir.ActivationFunctionType.Copy,
                                     scale=dis_pt[:w, tt:tt + 1])
                nc.sync.dma_start(out=table[tt * P:tt * P + w, 0:D],
                                  in_=xb[:w, :])

            # ---- layers
            for rep in range(reps):
                for li in range(3):
                    last = (li == 2)
                    Wl = W_sb[li]
                    # aggregation + z, chunked
                    pend = []          # per-window node-major U psum tiles
                    gdone = 0

                    def flush_group(pend_tiles, gidx):
                        # pend_tiles: list of (t, w, um_tile)
                        ptru = ps_tru.tile([D, RW], bf16, tag="tru",
                                           space="PSUM")
                        gw = 0
                        for (tt_, w_, um_) in pend_tiles:
                            nc.tensor.transpose(ptru[:, gw:gw + w_],
                                                um_[:w_, :], identP[:w_, :w_])
                            gw += w_
                        ut = wpool.tile([D, RW], bf16, tag="ut")
                        nc.vector.tensor_copy(ut[:, :gw], ptru[:, :gw])
                        psz = ps_z.tile([D, RW], f32, tag="zt", space="PSUM")
                        nc.tensor.matmul(psz[:, :gw], lhsT=Wl[:],
                                         rhs=ut[:, :gw],
                                         start=True, stop=True)
                        zt = wpool.tile([D, RW], bf16, tag="ztile")
                        nc.scalar.activation(
                            zt[:, :gw], psz[:, :gw],
                            mybir.ActivationFunctionType.Copy,
                            accum_out=sums[:, gidx:gidx + 1])
                        sq = wpool.tile([D, RW], f32, tag="sq")
                        nc.vector.tensor_tensor(
                            out=sq[:, :gw], in0=psz[:, :gw], in1=psz[:, :gw],
                            op=mybir.AluOpType.mult)
                        nc.vector.reduce_sum(sums2[:, gidx:gidx + 1],
                                             sq[:, :gw],
                                             axis=mybir.AxisListType.X)
                        g0 = pend_tiles[0][0] * P
                        nc.vector.dma_start(out=z_dram[:, g0:g0 + gw],
                                            in_=zt[:, :gw])

                    for (t0, t1) in chunks:
                        nb0c = cum0[t1] - cum0[t0]
                        nb1c = cum1[t1] - cum1[t0]
                        nbsc = sb[t1] - sb[t0]
                        S_t = spool.tile([P, CBS, P], fp8, tag="S")
                        nc.sync.dma_start(out=S_t[:, :nbsc, :],
                                          in_=S_in[:, sb[t0]:sb[t1], :])
                        ixoff = (cum0[t0] + cum1[t0]) * 8
                        ixw = (nb0c + nb1c) * 8
                        ixc = ipool.tile([P, (CB0 + CB1) * 8], i16, tag="ixc")
                        nc.sync.dma_start(
                            out=ixc[:, :ixw],
                            in_=idxc_in[:, ixoff:ixoff + ixw])
                        m0 = mpool.tile([P, CB0, TROW], bf16, tag="m0")
                        if nb0c:
                            nc.gpsimd.dma_gather(
                                m0[:, :nb0c, :], table[:HALF, :],
                                ixc[:, :nb0c * 8],
                                nb0c * P, nb0c * P, TROW,
                                single_packet=False)
                        m1 = mpool.tile([P, CB1, TROW], bf16, tag="m1")
                        if nb1c:
                            nc.gpsimd.dma_gather(
                                m1[:, :nb1c, :], table[HALF:ntp, :],
                                ixc[:, nb0c * 8:ixw],
                                nb1c * P, nb1c * P, TROW,
                                single_packet=False)
                        for tt in range(t0, t1):
                            w = tsz[tt]
                            nblocks = nb[tt][0] + nb[tt][1]
                            ps = ps_acc.tile([P, D], f32, tag="acc",
                                             space="PSUM")
                            for i in range(nblocks):
                                if i < nb[tt][0]:
                                    rhs = m0[:, cum0[tt] - cum0[t0] + i, 0:D]
                                else:
                                    rhs = m1[:, cum1[tt] - cum1[t0]
                                             + (i - nb[tt][0]), 0:D]
                                lhs = S_t[:, sb[tt] - sb[t0] + i, :]
                                nc.tensor.matmul(
                                    ps[:], lhsT=lhs, rhs=rhs,
                                    start=(i == 0), stop=(i == nblocks - 1))
                            # U node-major = psum * dis (per-partition scale)
                            um = umpool.tile([P, D], bf16, tag="um")
                            nc.vector.tensor_tensor(
                                out=um[:w, :], in0=ps[:w, :],
                                in1=dis_pt[:w, tt:tt + 1].to_broadcast([w, D]),
                                op=mybir.AluOpType.mult)
                            pend.append((tt, w, um))
                            if len(pend) == GW:
                                flush_group(pend, gdone)
                                pend = []
                                gdone += 1
                    if pend:
                        flush_group(pend, gdone)
                        pend = []
                        gdone += 1

                    # ---- global BN stats (local reduction — no collective)
                    st = wpool.tile([D, 2], f32, tag="st")
                    nc.vector.reduce_sum(st[:, 0:1], sums[:],
                                         axis=mybir.AxisListType.X)
                    nc.vector.reduce_sum(st[:, 1:2], sums2[:],
                                         axis=mybir.AxisListType.X)
                    mu = wpool.tile([D, 1], f32, tag="mu")
                    nc.scalar.activation(mu[:], st[:, 0:1],
                                         mybir.ActivationFunctionType.Copy,
                                         scale=1.0 / n_nodes)
                    va = wpool.tile([D, 1], f32, tag="va")
                    nc.scalar.activation(va[:], st[:, 1:2],
                                         mybir.ActivationFunctionType.Copy,
                                         scale=1.0 / n_nodes)
                    mu2 = wpool.tile([D, 1], f32, tag="mu2")
                    nc.vector.tensor_tensor(out=mu2[:], in0=mu[:], in1=mu[:],
                                            op=mybir.AluOpType.mult)
                    nc.vector.tensor_tensor(out=va[:], in0=va[:], in1=mu2[:],
                                            op=mybir.AluOpType.subtract)
                    nc.scalar.activation(va[:], va[:],
                                         mybir.ActivationFunctionType.Sqrt,
                                         bias=eps_sb[:])
                    nc.vector.reciprocal(va[:], va[:])
                    saff = wpool.tile([D, 1], f32, tag="saff")
                    nc.vector.tensor_tensor(out=saff[:], in0=gamma_sb[:],
                                            in1=va[:], op=mybir.AluOpType.mult)
                    tsh_ = wpool.tile([D, 1], f32, tag="tsh")
                    nc.vector.tensor_tensor(out=tsh_[:], in0=mu[:], in1=saff[:],
                                            op=mybir.AluOpType.mult)
                    nc.vector.tensor_tensor(out=tsh_[:], in0=beta_sb[:],
                                            in1=tsh_[:],
                                            op=mybir.AluOpType.subtract)

                    # ---- activation phase per group
                    first_seen = set()
                    for gr in range(ngrp):
                        a = gr * GW * P
                        b = min((gr + 1) * GW * P, n_nodes)
                        gw = b - a
                        zt2 = wpool.tile([D, RW], bf16, tag="zt2")
                        nc.vector.dma_start(out=zt2[:, :gw],
                                            in_=z_dram[:, a:a + gw])
                        at = wpool.tile([D, RW], bf16, tag="at")
                        nc.scalar.activation(at[:, :gw], zt2[:, :gw],
                                             mybir.ActivationFunctionType.Relu,
                                             bias=tsh_[:], scale=saff[:])
                        if not last:
                            tts = list(range(gr * GW,
                                             min((gr + 1) * GW, ntile)))
                            full = all(tsz[tt] == P for tt in tts)
                            wr4 = wpool.tile([P, GW, D], bf16, tag="wr4")
                            for gi, tt in enumerate(tts):
                                w = tsz[tt]
                                o = tt * P - a
                                ptr = ps_tru.tile([P, D], bf16,
                                                  tag="trp", space="PSUM")
                                nc.tensor.transpose(ptr[:w, :],
                                                    at[:, o:o + w],
                                                    identP[:D, :D])
                                nc.scalar.activation(
                                    wr4[:w, gi, :], ptr[:w, :],
                                    mybir.ActivationFunctionType.Copy,
                                    scale=dis_pt[:w, tt:tt + 1])
                                if not full:
                                    nc.vector.dma_start(
                                        out=table[tt * P:tt * P + w, 0:D],
                                        in_=wr4[:w, gi, :])
                            if full:
                                tv = table[a:a + GW * P, 0:D].rearrange(
                                    "(g p) d -> p g d", p=P)
                                nc.vector.dma_start(out=tv,
                                                    in_=wr4[:, :len(tts), :])
                        elif rep == reps - 1:
                            # pool graph segments on the fly
                            for (gr_, s0, s1, g) in cfg["pool_segs"]:
                                if gr_ != gr:
                                    continue
                                tmp = wpool.tile([D, 1], f32, tag="ptmp")
                                nc.vector.reduce_max(
                                    tmp[:], at[:, s0:s1],
                                    axis=mybir.AxisListType.X)
                                if g not in first_seen:
                                    first_seen.add(g)
                                    nc.vector.tensor_copy(emb[:, g:g + 1],
                                                          tmp[:])
                                else:
                                    nc.vector.tensor_tensor(
                                        out=emb[:, g:g + 1],
                                        in0=emb[:, g:g + 1], in1=tmp[:],
                                        op=mybir.AluOpType.max)

            # ---- head (all graphs, every core)
            emb_bf = wpool.tile([D, n_graphs], bf16, tag="embbf")
            nc.vector.tensor_copy(emb_bf[:], emb[:])
            ph_full = ps_z.tile([D, RW], f32, tag="zt", space="PSUM")
            ph = ph_full[:, :n_graphs]
            nc.tensor.matmul(ph, lhsT=l1w_sb[:], rhs=emb_bf[:],
                             start=True, stop=True)
            h1 = wpool.tile([D, n_graphs], bf16, tag="h1")
            nc.scalar.activation(h1[:], ph,
                                 mybir.ActivationFunctionType.Relu,
                                 bias=l1b_sb[:])
            po_full = ps_z.tile([D, RW], f32, tag="zt", space="PSUM")
            po = po_full[:ncls, :n_graphs]
            nc.tensor.matmul(po, lhsT=l2w_sb[:], rhs=h1[:],
                             start=True, stop=True)
            osb = wpool.tile([ncls, n_graphs], f32, tag="osb")
            nc.scalar.activation(osb[:], po,
                                 mybir.ActivationFunctionType.Identity,
                                 bias=l2b_sb[:])
            nc.sync.dma_start(out=out[:, :].rearrange("g c -> c g"), in_=osb[:])

    nc.compile()
    return nc


# ---------------------------------------------------------------- entry point

_CACHE = {}


def _get_built(cfg_key, cfg, reps, n_devices=N_CORES):
    key = (cfg_key, reps, n_devices)
    if key not in _CACHE:
        _CACHE[key] = _build(cfg, reps=reps, n_devices=n_devices)
    return _CACHE[key]


def _make_in_maps(cfg, data, x, inputs, n_devices=N_CORES):
    ncls = cfg["n_classes"]
    W_bf = [np.asarray(inputs[k], np.float32).astype(ml_dtypes.bfloat16)
            for k in ("W1", "W2", "W3")]
    m = {
        "x": x.astype(np.float32),
        "S": data["S"],
        "idx0": data["idx0"],
        "idx1": data["idx1"],
        "deg_pt": data["deg_pt"],
        "W1": W_bf[0], "W2": W_bf[1], "W3": W_bf[2],
        "gamma": np.asarray(inputs["gamma"], np.float32).reshape(D, 1),
        "beta": np.asarray(inputs["beta"], np.float32).reshape(D, 1),
        "lin1w": np.asarray(inputs["lin1_w"],
                            np.float32).astype(ml_dtypes.bfloat16),
        "lin1b": np.asarray(inputs["lin1_b"], np.float32).reshape(D, 1),
        "lin2w": np.asarray(inputs["lin2_w"],
                            np.float32).astype(ml_dtypes.bfloat16),
        "lin2b": np.asarray(inputs["lin2_b"], np.float32).reshape(ncls, 1),
    }
    return [m for _ in range(n_devices)]


def kernel(x, edge_index, batch, W1, b1, W2, b2, W3, b3, gamma, beta,
           lin1_w, lin1_b, lin2_w, lin2_b, _reps=1, _ndev=N_CORES):
    x = np.asarray(x, np.float32)
    edge_index = np.asarray(edge_index)
    batch = np.asarray(batch)
    n_nodes, d = x.shape
    ncls = np.asarray(lin2_w).shape[1]
    assert d == D

    cfg, data = _prep(x, edge_index, batch, ncls)

    # NOTE: b1/b2/b3 cancel inside BatchNorm (mean subtraction) - unused.
    in_maps = _make_in_maps(cfg, data, x, {
        "W1": W1, "W2": W2, "W3": W3, "gamma": gamma, "beta": beta,
        "lin1_w": lin1_w, "lin1_b": lin1_b, "lin2_w": lin2_w,
        "lin2_b": lin2_b}, n_devices=_ndev)

    cfg_key = (n_nodes, edge_index.shape[1], ncls, cfg["NBLK"])
    nc = _get_built(cfg_key, cfg, _reps, _ndev)
    res = run_bass_kernel_spmd(nc, in_maps, core_ids=list(range(_ndev)))
    return np.asarray(res.results[0]["out"]).astype(np.float32)
